# revision 66
# baseline (speedup 1.0000x reference)
"""CPAMDec attention-decoder kernel for 8 Trainium2 NeuronCores.

Reference computation (per batch n of N=8):
    q  = x_n^T @ wq^T + bq          (HW=4096, C4=128)
    k  = y_n @ wk^T + bk            (K=32, C4=128)
    v  = y_n @ wv^T + bv            (K=32, C=512)
    attn = softmax(q @ k^T, axis=-1)        (HW, K)
    out = scale * (v^T @ attn^T) + x_n      (C, HW)

Sharding: pure data parallel - core i computes batch i.

This version is DMA-traffic-optimized: the tolerance (2e-2) leaves room
to ship x, out and all params as fp16, halving HBM bytes (the dominant
cost: the fp32 version is DMA-saturated for its entire 80us runtime).
Host pre-arranges x and out in partition-major layout so every bulk DMA
line is 4KB contiguous (128 descriptors per 512KB chunk instead of 512).

All 8 input chunks are prefetched up front; since the PE's work (~17us
at full clock) exceeds the per-chunk DMA cadence, the PE stream stays
dense once started, which keeps the HAM clock gate at 2.4 GHz (any PE
idle gap resets the clock to 0.65 GHz - the fp32 baseline lost 2x here).

Bias folding:
  - bq contributes a per-key bias e_b[j] = sum_o bq[o]*k'[j,o], applied
    inside the exp() activation (exact algebra); a constant -6 shift is
    folded in as well so exp() stays in fp16 range (softmax-invariant).
  - bv is folded into v via an extra rank-1 matmul row, and scale s into
    v_sb = s*(v+bv), so the output stage is a plain residual add.
"""

import sys

sys.path.insert(0, "/opt/trn_rl_repo")

import numpy as np

import concourse.bacc as bacc
import concourse.mybir as mybir
import concourse.tile as tile
from concourse.bass_utils import run_bass_kernel_spmd

F32 = mybir.dt.float32
F16 = mybir.dt.float16
AF = mybir.ActivationFunctionType

N, C, H, W, K = 8, 512, 64, 64, 32
HW = H * W            # 4096
C4 = C // 4           # 128
PC = 512              # free-dim chunk (1 PSUM bank of fp32)
NPC = HW // PC        # 8 chunks
KC = C // 128         # 4 contraction chunks
CT = C // 128         # 4 output row-tiles
CW = KC * PC          # 2048 elements per chunk per partition
PKW = 12 * 128 + C + 2 + K  # packed consts (wq|yt|wkt|bv|bq|bk|ident32)
ESHIFT = -6.0         # exp shift: keeps exp() outputs in fp16 range
WARMUP = 5            # PE busy-bridge matmuls (preamble-end -> first q)


def _emit(nc, tc):
    sync = nc.sync

    with (
        tc.tile_pool(name="const", bufs=1) as cst,
        tc.tile_pool(name="xbuf", bufs=1) as xp,
        tc.tile_pool(name="work", bufs=3) as wk_pool,
        tc.tile_pool(name="ps", bufs=2, space="PSUM") as ps,
    ):
        # ---- constant loads (scalar ring — just 2 triggers, then the
        # ACT queue is free for compute). pk packs wq(4)/yt(4)/wkt(4)
        # [128,128] tiles plus the replicated bv row and the bq/bk
        # columns, so there are no tiny-descriptor const DMAs (a [C4,1]
        # fp32 load is 128 4-byte descriptors that straggle for ~10us
        # behind the bulk streams).
        pk = cst.tile([128, PKW], F16, name="pk", tag="pk")
        nc.scalar.dma_start(pk[:], nc.t.pk[:])
        wvp = cst.tile([128, KC * C], F16, name="wvp", tag="wvp")
        nc.scalar.dma_start(wvp[:], nc.t.wvp[:])
        s_bc32 = cst.tile([K, 1], F32, name="s_bc32", tag="s_bc32")
        nc.gpsimd.dma_start(
            s_bc32[:], nc.t.s[:].partition_broadcast(K).squeeze(-1))

        wq_o = pk[:, 0:C]               # [C4, C] wq (o on partitions)

        def yt_t(k):
            return pk[:, (4 + k) * 128:(5 + k) * 128]

        def wkt_t(k):
            return pk[:, (8 + k) * 128:(9 + k) * 128]

        bv_mov = pk[0:1, 1536:1536 + C]     # [1, C] bv row (partition 0)
        bq_col = pk[:, 2048:2049]           # [C4, 1] bq column
        bk_col = pk[:, 2049:2050]           # [C4, 1] bk column
        ident32 = pk[0:K, 2050:2050 + K]    # [K, K] identity

        def wv_t(k):
            return wvp[:, k * C:(k + 1) * C]

        # memset-backed constants (no DMA dependency -> early warm-up)
        ones32 = cst.tile([K, 128], F16, name="ones32", tag="ones32")
        nc.gpsimd.memset(ones32[:], 1.0)
        onesk = cst.tile([1, K], F16, name="onesk", tag="onesk")
        nc.gpsimd.memset(onesk[:], 1.0)
        dmy_m = cst.tile([K, PC], F16, name="dmy_m", tag="dmy_m")
        nc.gpsimd.memset(dmy_m[:], 0.0)

        # ---- x prefetch: all up front on the sync ring (store triggers
        # queue behind them). Chunks 0,1 load singly (fast pipeline
        # start); the rest in 1MB pairs to cut trigger count.
        xs = [None] * NPC
        for pc in (0, 1):
            t = xp.tile([128, CW], F16, name=f"xs{pc}", tag=f"xs{pc}")
            sync.dma_start(t[:], nc.t.x16[:, pc * CW:(pc + 1) * CW])
            xs[pc] = t
        for pc in (2, 4, 6):
            t = xp.tile([128, 2 * CW], F16, name=f"xs{pc}", tag=f"xs{pc}")
            sync.dma_start(t[:], nc.t.x16[:, pc * CW:(pc + 2) * CW])
            xs[pc] = t[:, 0:CW]
            xs[pc + 1] = t[:, CW:2 * CW]

        # ---- PE warm-up: HAM clock gate needs ~3.4us of sustained ----
        # matmul activity to unthrottle 0.65 -> 2.4 GHz.
        dmy_ps = ps.tile([128, PC], F32, name="dmy_ps", tag="q", bufs=1)
        for _ in range(WARMUP):
            nc.tensor.matmul(dmy_ps[:], ones32[:], dmy_m[:],
                             start=True, stop=True)

        pro = {}

        def emit_prologue():
            # kT (with bk), v (s*(v+bv)), e_b — emitted after stage_q(0)
            # so q(0) leads the PE queue behind the warm-up.
            kt_ps = ps.tile([C4, 4 * K], F32, name="kt_ps", tag="e", bufs=2)
            for k in range(KC):
                nc.tensor.matmul(kt_ps[:], wkt_t(k), yt_t(k),
                                 start=(k == 0), stop=(k == KC - 1))
            ktb4 = cst.tile([C4, 4 * K], F16, name="ktb4", tag="ktb4")
            nc.scalar.activation(out=ktb4[:], in_=kt_ps[:], func=AF.Identity,
                                 bias=bk_col, scale=1.0)

            # fused energy weights: M_k[c', j] = sum_o wq[o, 128k+c']*k'[j, o]
            # so the energy matmul reads x directly (no per-chunk q pass
            # or PSUM->SBUF q copy).
            m_ps = ps.tile([128, 4 * 128], F32, name="m_ps", tag="q",
                           bufs=1)
            for k in range(KC):
                nc.tensor.matmul(m_ps[:, k * 128:(k + 1) * 128],
                                 wq_o[:, k * 128:(k + 1) * 128], ktb4[:],
                                 start=True, stop=True)
            m_sb = cst.tile([128, 4 * 128], F16, name="m_sb", tag="m_sb")
            nc.scalar.activation(out=m_sb[:], in_=m_ps[:], func=AF.Copy,
                                 scale=1.0)
            pro['m_sb'] = m_sb

            v_ps = ps.tile([K, C], F32, name="v_ps", tag="s", bufs=1)
            for k in range(KC):
                nc.tensor.matmul(v_ps[:], yt_t(k)[:, 0:K], wv_t(k),
                                 start=(k == 0), stop=False)
            # rank-1 bias row: v += 1 * bv  (exact)
            nc.tensor.matmul(v_ps[:], onesk[:], bv_mov,
                             start=False, stop=True)
            v_sb = cst.tile([K, C], F16, name="v_sb", tag="v_sb")
            nc.scalar.activation(out=v_sb[:], in_=v_ps[:], func=AF.Copy,
                                 bias=0.0, scale=s_bc32[:])
            # partition-stack via PE identity matmuls (column-offset
            # tile_position writes band ct) — much faster than serial
            # SWDGE copies, which sat on the ramp critical path:
            # vstack[32*ct + j, m] = v_sb[j, 128*ct + m]
            vs_ps_t = ps.tile([128, PC], F32, name="vs_ps", tag="o",
                              bufs=4)
            vs_ps = vs_ps_t[:, 0:128]
            for ct in range(CT):
                nc.tensor.matmul(vs_ps[32 * ct:32 * (ct + 1), :],
                                 ident32, v_sb[:, 128 * ct:128 * (ct + 1)],
                                 start=True, stop=True,
                                 tile_position=(0, 32 * ct))
            vstack = cst.tile([128, 128], F16, name="vstack", tag="vstack")
            nc.scalar.activation(out=vstack[:], in_=vs_ps[:], func=AF.Copy,
                                 scale=1.0)

            eb_ps = ps.tile([4 * K, 1], F32, name="eb_ps", tag="o", bufs=4)
            nc.tensor.matmul(eb_ps[:], ktb4[:], bq_col, start=True,
                             stop=True)
            e_b4 = cst.tile([4 * K, 1], F32, name="e_b4", tag="e_b4")
            nc.scalar.activation(out=e_b4[:], in_=eb_ps[:],
                                 func=AF.Copy, bias=ESHIFT, scale=1.0)
            pro.update(ktb4=ktb4, vstack=vstack, e_b4=e_b4)

        # ------------- software-pipelined main loop over column chunks
        #   step:   q(step)   e/exp(step-1)   sum/rec/mul(step-2)
        #           finals/add/store(step-3)
        expts = [None] * NPC
        attns = [None] * NPC

        def stage_energy(pc):
            # fused q+energy: e = M^T x straight from the x chunk
            xt = xs[pc]
            e_ps = ps.tile([128, PC], F32, name=f"e_ps{pc}", tag="e", bufs=2)
            for k in range(KC):
                nc.tensor.matmul(e_ps[:], pro['m_sb'][:, k * 128:(k + 1) * 128],
                                 xt[:, k * PC:(k + 1) * PC],
                                 start=(k == 0), stop=(k == KC - 1))
            expt = wk_pool.tile([128, PC], F16, name="expt", tag="expt",
                                bufs=4)
            nc.scalar.activation(out=expt[:], in_=e_ps[:], func=AF.Exp,
                                 bias=pro['e_b4'][:], scale=1.0)
            expts[pc] = expt

        def stage_softmax(pc):
            s_ps = ps.tile([128, PC], F32, name=f"s_ps{pc}", tag="s", bufs=1)
            nc.tensor.matmul(s_ps[:], ones32[:], expts[pc][0:K, :],
                             start=True, stop=True)
            rec = wk_pool.tile([128, PC], F32, name="rec", tag="rec", bufs=4)
            nc.vector.reciprocal_approx_fast(out=rec[:], in_=s_ps[:])
            # downcast on ACT (it has slack now) so the multiply runs in
            # the DVE's 2x all-fp16 mode
            rec16 = wk_pool.tile([128, PC], F16, name="rec16", tag="rec16",
                                 bufs=4)
            # split the downcast across ACT and DVE halves: ACT is the
            # pacing engine, so offload half of its cheapest op
            nc.scalar.activation(out=rec16[:, 0:PC // 2],
                                 in_=rec[:, 0:PC // 2], func=AF.Copy,
                                 scale=1.0)
            nc.vector.tensor_copy(rec16[:, PC // 2:PC], rec[:, PC // 2:PC])
            attn = wk_pool.tile([128, PC], F16, name="attn", tag="attn",
                                bufs=4)
            nc.vector.tensor_mul(attn[:], expts[pc][:], rec16[:])
            attns[pc] = attn

        def stage_out(pc):
            xt = xs[pc]
            attn = attns[pc]
            osb = wk_pool.tile([128, CT * PC], F16, name="osb", tag="osb",
                               bufs=4)
            # ct order 2,3,0,1: the slow ACT->Pool half starts first.
            # Last chunk goes all-DVE (Pool's ~1.3us adds would stretch
            # the drain tail).
            pool_cts = () if pc == NPC - 1 else (2, 3)
            for ct in (2, 3, 0, 1):
                o_ps = ps.tile([128, PC], F32, name=f"o_ps{pc}_{ct}",
                               tag="o", bufs=4)
                nc.tensor.matmul(o_ps[:],
                                 pro['vstack'][32 * ct:32 * (ct + 1), :],
                                 attn[32 * ct:32 * (ct + 1), :],
                                 start=True, stop=True,
                                 tile_position=(32 * ct, 0))
                if ct in pool_cts:
                    # Pool can't read PSUM: ACT downcasts, Pool adds
                    oc = wk_pool.tile([128, PC], F16, name=f"oc{ct}",
                                      tag=f"oc{ct}", bufs=3)
                    nc.scalar.activation(out=oc[:], in_=o_ps[:],
                                         func=AF.Copy, scale=1.0)
                    nc.gpsimd.tensor_add(osb[:, ct * PC:(ct + 1) * PC],
                                         oc[:],
                                         xt[:, ct * PC:(ct + 1) * PC])
                else:
                    # DVE adds straight from PSUM
                    nc.vector.tensor_add(osb[:, ct * PC:(ct + 1) * PC],
                                         o_ps[:],
                                         xt[:, ct * PC:(ct + 1) * PC])
            # two half-stores: the DVE half doesn't wait on the Pool half
            half = CT * PC // 2
            sync.dma_start(nc.t.out16[:, pc * CW + half:(pc + 1) * CW],
                           osb[:, half:])
            sync.dma_start(nc.t.out16[:, pc * CW:pc * CW + half],
                           osb[:, 0:half])

        emit_prologue()
        stage_energy(0)
        for step in range(1, NPC + 2):
            if 0 <= step - 2 < NPC:
                stage_out(step - 2)
            if step < NPC:
                stage_energy(step)
            if 0 <= step - 1 < NPC:
                stage_softmax(step - 1)


class _T:
    """Attribute access to declared dram params."""
    def __init__(self):
        self.__dict__ = {}


_NC_CACHE = []


def _build():
    if _NC_CACHE:
        return _NC_CACHE[0]
    nc = bacc.Bacc(target_bir_lowering=False)
    nc.t = _T()
    t = nc.t
    t.x16 = nc.declare_dram_parameter("x16", [128, NPC * CW], F16,
                                      isOutput=False)
    t.pk = nc.declare_dram_parameter("pk", [128, PKW], F16,
                                     isOutput=False)
    t.wvp = nc.declare_dram_parameter("wvp", [128, KC * C], F16,
                                      isOutput=False)
    t.s = nc.declare_dram_parameter("s", [1, 1], F32, isOutput=False)
    t.out16 = nc.declare_dram_parameter("out16", [128, NPC * CW], F16,
                                        isOutput=True)
    with tile.TileContext(nc) as tc:
        _emit(nc, tc)
    nc.finalize()
    _NC_CACHE.append(nc)
    return nc


def _in_maps(x, y, wq, bq, wk, bk, wv, bv, scale):
    f16 = np.float16
    # x: (N,C,H,W) -> per-core [128, NPC*KC*PC] partition-major fp16,
    # so every chunk DMA line is 4KB contiguous per partition.
    x16 = (np.asarray(x, dtype=np.float32)
           .reshape(N, KC, 128, NPC, PC)
           .transpose(0, 2, 3, 1, 4)
           .reshape(N, 128, NPC * CW)
           .astype(f16))
    # packed const tile: wq(4)|yt(4)|wkt(4) [128,128] tiles along free
    # dim, then the bv row replicated across partitions and the bq/bk
    # columns.
    wq_p = np.ascontiguousarray(np.float32(wq))  # [C4, C] o-major
    yt_p = (np.tile(np.transpose(np.float32(y), (0, 2, 1)), (1, 1, 4))
            .reshape(N, KC, 128, 4 * K).transpose(0, 2, 1, 3)
            .reshape(N, 128, 4 * 4 * K))
    wkt_p = np.float32(wk).T.reshape(KC, 128, C4).transpose(1, 0, 2)
    wkt_p = wkt_p.reshape(128, 4 * C4)
    bv_rep = np.broadcast_to(np.float32(bv).reshape(1, C), (128, C))
    bq_c = np.float32(bq).reshape(C4, 1)
    bk_c = np.float32(bk).reshape(C4, 1)
    id32 = np.zeros((128, K), dtype=np.float32)
    id32[:K, :] = np.eye(K, dtype=np.float32)
    pk_n = [
        np.concatenate([wq_p, yt_p[i], wkt_p, bv_rep, bq_c, bk_c, id32],
                       axis=1).astype(f16)
        for i in range(N)
    ]
    wvp = (np.float32(wv).T.reshape(KC, 128, C).transpose(1, 0, 2)
           .reshape(128, KC * C).astype(f16))
    s = np.ascontiguousarray(scale, dtype=np.float32).reshape(1, 1)
    return [
        {
            "x16": np.ascontiguousarray(x16[i]), "pk": pk_n[i], "wvp": wvp,
            "s": s,
        }
        for i in range(N)
    ]


def _run(inputs, **kwargs):
    nc = _build()
    return run_bass_kernel_spmd(nc, _in_maps(**inputs),
                                core_ids=list(range(N)), **kwargs)


def kernel(**inputs) -> np.ndarray:
    res = _run(inputs)
    # out16 [128, NPC*CT*PC] fp16 partition-major -> (C, HW) fp32
    out = np.stack([
        res.results[i]["out16"]
        .reshape(128, NPC, CT, PC)
        .transpose(2, 0, 1, 3)
        .reshape(C, HW)
        for i in range(N)
    ]).astype(np.float32)
    return out.reshape(N, C, H, W)
